# revision 21
# baseline (speedup 1.0000x reference)
"""Cross-attention kernel for Trainium2, 8 NeuronCores.

Problem (full shapes): B=4, Lq=Lk=2048, D(in)=D(out)=1024, fp32.
    q = query @ Wq + bq ; k = key @ Wk + bk ; v = value @ Wv + bv
    out = softmax(q k^T / sqrt(1024)) v

Sharding: 8 cores = (batch b, half h of Lq). Each core computes a
[1024, 1024] slice of the output for batch b, query rows
[h*1024, (h+1)*1024).

Algebraic restructure (removes the duplicated K/V projections):
  s_ij = q_i . k_j  with  q_i = x_i Wq + bq,  k_j = y_j Wk + bk
  (x = query_input, y = key_input).  The q_i.bk term is constant along
  each softmax row -> drops under softmax.  So
      S = x (Wq Wk^T) Y^T + (Wk bq)^T Y^T-rows
  where A = Wq @ Wk^T and bqk = Wk @ bq are host-precomputed
  weights-only transforms: ONE projection of x (1024 rows) replaces
  projecting both Q (1024 rows) and K (2048 rows).  Likewise, softmax
  rows sum to 1, so
      out_i = sum_j p_ij (v_j Wv + bv) = (P V) Wv + bv
  computes Z = P @ value_input first, then one Z @ Wv on 1024 rows
  instead of projecting V on 2048 rows.  Per-core matmul work drops
  from ~19.3 GFLOP to ~12.8 GFLOP with no cross-core traffic.

Per-core phases (P=128 partitions, all matmul operands bf16):
  P1  qkT[e, lq]   = (x @ Wqk + bqk)^T        128 MM N=512
  P3  per 256-row block: ST[lk,lq] = kS^T qkT (8 MM N=256 per chunk),
      exp via scalar engine (no max-subtraction: |s/32| < ~3),
      row sums r from a PE matmul with a ones column,
      ZnumT[d, lq] accumulated over key chunks (rhs = exp tile).
  P4  out = (ZnumT^T @ Wv) * (1/r) + bv       128 MM N=512
"""

import os
import sys

sys.path.insert(0, "/opt/trn_rl_repo")

from contextlib import ExitStack

import numpy as np

import concourse.bass as bass
import concourse.tile as tile
from concourse import bacc, mybir
from concourse.bass_utils import run_bass_kernel_spmd

P = 128
B, LQ, LK, D = 4, 2048, 2048, 1024
NCORES = 8
LQS = LQ * B // NCORES  # 1024 query rows per core
KCH = D // P  # 8 contraction chunks
DOT = D // P  # 8 output-feature tiles
LKT = LK // P  # 16 key tiles
PBLK = 512  # projection matmul free dim
ABLK = 256  # attention lq block (2 lq tiles)
NBLK = LQS // ABLK  # 4
SCALE = 1.0 / 32.0  # 1/sqrt(D)

F32 = mybir.dt.float32
BF = mybir.dt.bfloat16


def _host_bf16():
    import ml_dtypes

    return np.dtype(ml_dtypes.bfloat16)


def build_program(repeat=1):
    nc = bacc.Bacc("TRN2", target_bir_lowering=False, debug=False)

    q_inT = nc.dram_tensor("q_inT", [D, LQS], BF, kind="ExternalInput").ap()
    k_inT = nc.dram_tensor("k_inT", [D, LK], BF, kind="ExternalInput").ap()
    v_in = nc.dram_tensor("v_in", [LK, D], BF, kind="ExternalInput").ap()
    # Wqk = Wq @ Wk^T and bqk = Wk @ bq are host-precomputed (weights-
    # only transforms), so the q/k projection pair is a single GEMM.
    Wqk = nc.dram_tensor("Wqk", [D, D], BF, kind="ExternalInput").ap()
    Wv = nc.dram_tensor("Wv", [D, D], BF, kind="ExternalInput").ap()
    bqk = nc.dram_tensor("bqk", [D], F32, kind="ExternalInput").ap()
    bv = nc.dram_tensor("bv", [D], F32, kind="ExternalInput").ap()
    out = nc.dram_tensor("out", [LQS, D], F32, kind="ExternalOutput").ap()

    q_t = q_inT.rearrange("(o p) l -> p o l", p=P)
    k_t = k_inT.rearrange("(o p) l -> p o l", p=P)
    v_t = v_in.rearrange("(t p) d -> p t d", p=P)
    Wqk_t = Wqk.rearrange("(o p) n -> p o n", p=P)
    Wv_t = Wv.rearrange("(o p) n -> p o n", p=P)

    with tile.TileContext(nc) as tc, ExitStack() as ctx:
        # PSUM is 8 banks; pools reserve bank-granular: 2+2+2+2.
        psum_p = ctx.enter_context(tc.tile_pool(name="psum_p", bufs=2, space="PSUM"))
        psum_st = ctx.enter_context(tc.tile_pool(name="psum_st", bufs=2, space="PSUM"))
        psum_zt = ctx.enter_context(tc.tile_pool(name="psum_zt", bufs=2, space="PSUM"))
        psum_r = ctx.enter_context(tc.tile_pool(name="psum_r", bufs=2, space="PSUM"))
        singles = ctx.enter_context(tc.tile_pool(name="singles", bufs=1))
        wq_pool = ctx.enter_context(tc.tile_pool(name="wq_pool", bufs=1))
        wv_pool = ctx.enter_context(tc.tile_pool(name="wv_pool", bufs=1))
        ks_pool = ctx.enter_context(tc.tile_pool(name="ks_pool", bufs=1))
        v_pool = ctx.enter_context(tc.tile_pool(name="v_pool", bufs=1))
        qk_pool = ctx.enter_context(tc.tile_pool(name="qk_pool", bufs=1))
        zn_pool = ctx.enter_context(tc.tile_pool(name="zn_pool", bufs=1))
        pt_pool = ctx.enter_context(tc.tile_pool(name="pt_pool", bufs=2))
        osb_pool = ctx.enter_context(tc.tile_pool(name="osb_pool", bufs=2))

        # ---- constants -------------------------------------------------
        bqk_sb = singles.tile([P, DOT], F32, name="bqk_sb")
        nc.sync.dma_start(bqk_sb[:], bqk.rearrange("(o p) -> p o", p=P))
        # bv broadcast to all 128 partitions (stride-0 partition read)
        bv_rep = singles.tile([P, D], F32, name="bv_rep")
        bv_bcast = bass.AP(tensor=bv.tensor, offset=bv.offset, ap=[[0, P], *bv.ap])
        nc.gpsimd.dma_start(bv_rep[:], bv_bcast)
        ones_sb = singles.tile([P, 2], BF, name="ones_sb")
        nc.vector.memset(ones_sb[:], 1.0)

        for _rep in range(repeat):
            one_pass(nc, tc, psum_p, psum_st, psum_zt, psum_r,
                     wq_pool, wv_pool, ks_pool, v_pool, qk_pool,
                     zn_pool, pt_pool, osb_pool,
                     bqk_sb, bv_rep, ones_sb,
                     q_t, k_t, v_t, Wqk_t, Wv_t, out)

    nc.compile()
    return nc


def one_pass(nc, tc, psum_p, psum_st, psum_zt, psum_r,
             wq_pool, wv_pool, ks_pool, v_pool, qk_pool,
             zn_pool, pt_pool, osb_pool,
             bqk_sb, bv_rep, ones_sb,
             q_t, k_t, v_t, Wqk_t, Wv_t, out):
    # ---- resident loads (distributed over trigger queues so they all
    # fire as soon as their WAR hazards clear) -------------------------
    Wqk_sb = wq_pool.tile([P, KCH, D], BF, tag="wq", name="Wqk_sb")
    for o in range(KCH):
        nc.sync.dma_start(Wqk_sb[:, o], Wqk_t[:, o])
    Wv_sb = wv_pool.tile([P, KCH, D], BF, tag="wv", name="Wv_sb")
    for o in range(KCH):
        nc.gpsimd.dma_start(Wv_sb[:, o], Wv_t[:, o])
    kS_sb = ks_pool.tile([P, KCH, LK], BF, tag="ks", name="kS_sb")
    for o in range(KCH):
        nc.sync.dma_start(kS_sb[:, o], k_t[:, o])
    v_sb = v_pool.tile([P, LKT, D], BF, tag="v", name="v_sb")
    for t in range(LKT):
        nc.gpsimd.dma_start(v_sb[:, t], v_t[:, t])

    qkT = qk_pool.tile([P, KCH, LQS], BF, tag="qk", name="qkT")

    with tc.tile_pool(name="qin_pool", bufs=1) as qin_pool:
        qin = qin_pool.tile([P, KCH, LQS], BF, tag="qin", name="qin")
        for o in range(KCH):
            nc.scalar.dma_start(qin[:, o], q_t[:, o])

        # ---- P1: qkT[e, lq] = (x @ Wqk + bqk)^T -----------------------
        for n in range(LQS // PBLK):
            for m in range(DOT):
                ps = psum_p.tile([P, PBLK], F32, tag="p", name="ps_qk")
                for k in range(KCH):
                    nc.tensor.matmul(
                        ps[:],
                        Wqk_sb[:, k, m * P : (m + 1) * P],
                        qin[:, k, n * PBLK : (n + 1) * PBLK],
                        start=(k == 0),
                        stop=(k == KCH - 1),
                    )
                nc.vector.tensor_scalar_add(
                    qkT[:, m, n * PBLK : (n + 1) * PBLK],
                    ps[:],
                    bqk_sb[:, m : m + 1],
                )

    # ---- P3: attention ------------------------------------------------
    znumT = zn_pool.tile([P, KCH, LQS], BF, tag="zn", name="znumT")
    rsb_all = zn_pool.tile([P, 2 * NBLK], F32, tag="rsb", name="rsb_all")
    for blk in range(NBLK):
        lq0 = blk * ABLK
        pt = pt_pool.tile([P, LKT, ABLK], BF, tag="pt", name="pt")
        r_ps = [
            psum_r.tile([P, 2], F32, tag="r", name=f"r_ps_{t}")
            for t in range(ABLK // P)
        ]

        def r_mms(c):
            # row sums via ones-column matmul; placed one chunk behind
            # the S matmuls so exp(c) hides under S(c+1) on the PE.
            for t in range(ABLK // P):
                nc.tensor.matmul(
                    r_ps[t][:],
                    pt[:, c, t * P : (t + 1) * P],
                    ones_sb[:],
                    start=(c == 0),
                    stop=(c == LKT - 1),
                )

        for c in range(LKT):
            st = psum_st.tile([P, ABLK], F32, tag="st", name="st")
            for e in range(KCH):
                nc.tensor.matmul(
                    st[:],
                    kS_sb[:, e, c * P : (c + 1) * P],
                    qkT[:, e, lq0 : lq0 + ABLK],
                    start=(e == 0),
                    stop=(e == KCH - 1),
                )
            nc.scalar.activation(
                pt[:, c], st[:], mybir.ActivationFunctionType.Exp, scale=SCALE
            )
            if c > 0:
                r_mms(c - 1)
        r_mms(LKT - 1)

        for dq in range(D // ABLK):  # quarters of D: 2 live zt tiles
            zt = [
                psum_zt.tile([P, ABLK], F32, tag="zt", name=f"zt_{j}")
                for j in range(ABLK // P)
            ]
            for c in range(LKT):
                for j in range(ABLK // P):
                    nc.tensor.matmul(
                        zt[j][:],
                        v_sb[:, c, dq * ABLK + j * P : dq * ABLK + (j + 1) * P],
                        pt[:, c],
                        start=(c == 0),
                        stop=(c == LKT - 1),
                    )
            for j in range(ABLK // P):
                nc.vector.tensor_scalar_add(
                    znumT[:, dq * (ABLK // P) + j, lq0 : lq0 + ABLK], zt[j][:], 0.0
                )

        for t in range(ABLK // P):
            i = blk * (ABLK // P) + t
            nc.vector.reciprocal(rsb_all[:, i : i + 1], r_ps[t][:, 0:1])

    # ---- P4: out = (ZnumT^T @ Wv) * (1/r) + bv ------------------------
    for blk in range(NBLK):
        for t in range(ABLK // P):
            i = blk * (ABLK // P) + t
            lq0 = blk * ABLK + t * P
            for dh in range(D // PBLK):
                ps = psum_p.tile([P, PBLK], F32, tag="p", name="ps_o")
                for k in range(KCH):
                    nc.tensor.matmul(
                        ps[:],
                        znumT[:, k, lq0 : lq0 + P],
                        Wv_sb[:, k, dh * PBLK : (dh + 1) * PBLK],
                        start=(k == 0),
                        stop=(k == KCH - 1),
                    )
                osb = osb_pool.tile([P, PBLK], F32, tag="osb", name="osb")
                nc.scalar.mul(osb[:], ps[:], rsb_all[:, i : i + 1])
                ob = osb_pool.tile([P, PBLK], F32, tag="ob", name="ob")
                nc.vector.tensor_add(
                    ob[:], osb[:], bv_rep[:, dh * PBLK : (dh + 1) * PBLK]
                )
                nc.sync.dma_start(
                    out[lq0 : lq0 + P, dh * PBLK : (dh + 1) * PBLK], ob[:]
                )


_program = None


def _get_program():
    global _program
    if _program is None:
        _program = build_program()
    return _program


def _make_in_maps(query_input, key_input, value_input, Wq, bq, Wk, bk, Wv, bv):
    bf = _host_bf16()
    f32 = np.float32
    Wq_f = np.asarray(Wq, f32)
    Wk_f = np.asarray(Wk, f32)
    bq_f = np.asarray(bq, f32)
    Wqk_h = np.ascontiguousarray((Wq_f @ Wk_f.T).astype(bf))
    bqk_h = Wk_f @ bq_f
    Wv_h = np.ascontiguousarray(np.asarray(Wv, bf))
    bv_h = np.asarray(bv, f32)
    in_maps = []
    kv_cache = {}
    for c in range(NCORES):
        b, h = divmod(c, 2)
        if b not in kv_cache:
            kv_cache[b] = (
                np.ascontiguousarray(np.asarray(key_input[b], bf).T),
                np.ascontiguousarray(np.asarray(value_input[b], bf)),
            )
        k_t, v_n = kv_cache[b]
        q_sh = np.asarray(query_input[b, h * LQS : (h + 1) * LQS, :], bf)
        in_maps.append(
            {
                "q_inT": np.ascontiguousarray(q_sh.T),
                "k_inT": k_t,
                "v_in": v_n,
                "Wqk": Wqk_h,
                "Wv": Wv_h,
                "bqk": bqk_h,
                "bv": bv_h,
            }
        )
    return in_maps


def run(in_maps, **kwargs):
    nc = _get_program()
    return run_bass_kernel_spmd(nc, in_maps, core_ids=list(range(NCORES)), **kwargs)


def kernel(query_input, key_input, value_input, Wq, bq, Wk, bk, Wv, bv):
    in_maps = _make_in_maps(
        query_input, key_input, value_input, Wq, bq, Wk, bk, Wv, bv
    )
    res = run(in_maps)
    out = np.empty((B, LQ, D), np.float32)
    for c in range(NCORES):
        b, h = divmod(c, 2)
        out[b, h * LQS : (h + 1) * LQS, :] = res.results[c]["out"]
    return out
